# revision 10
# baseline (speedup 1.0000x reference)
import math
import sys

sys.path.insert(0, "/opt/trn_rl_repo")

import numpy as np

N_CORES = 8
B, T, D = 65536, 64, 10
B_CORE = B // N_CORES  # 8192
P128 = 128

_cache = {}


def build(Pv, sharpv, Lv, b_core=B_CORE, tb=16):
    """Build + compile the per-core SPMD Bass program.

    Math: s_t = x_t + y_t, carry c_t (c_0 = 0), u_t = s_t + c_t:
      c_{t+1} = sigmoid(sharp*(u_t - 9.5))
      logits[t,d] = L*cos((2pi/P)*(u_t - d))

    On-chip state: h_t = tanh(sharp*(u_t-9.5)/2) = 2*c_{t+1}-1 (Tanh+Sin
    share an ACT table set) and r_t = s_t + 0.5*h_{t-1} = u_t - 0.5:
      r_t   = 0.5*h_{t-1} + s_t                  (one fused DVE op)
      h_t   = Tanh(r_t; scale=sharp/2, bias=-4.5*sharp)
      w_d   = wrap(r - (7+d))  into [-5, 5] by +-10   (digit units; one
              ADD_RANGE_WRAP per d in [0,5))
      logits[t,d]   = L*sin(delta*w_d)           (delta = 2pi/P; exact since
                      sin(delta*(r-7-d)) = cos(delta*(u-d)) mod 2pi)
      logits[t,d+5] = -logits[t,d]               (5*delta = pi for P=10)

    Perf notes (measured): DVE/POOL writes are ~1 cyc/elem even scattered,
    but only (mult,add)-style uops -- never op1=bypass (~14 cyc). DVE ts
    gets 2x on contiguous/run>=5 patterns. ACT needs contiguous in AND out
    (strided ~2 cyc). POOL is ~1 cyc contiguous, ~2 cyc strided reads.
    Layouts: R is t-major so chain writes are contiguous; wraps write a
    d-interleaved (t,n,d5) tile so sin runs contiguous-in-place and both
    +-L scatters are runs-of-5 -> runs-of-5.
    """
    import concourse.bacc as bacc
    import concourse.mybir as mybir
    import concourse.tile as tile

    fp32 = mybir.dt.float32
    i32 = mybir.dt.int32
    A = mybir.ActivationFunctionType
    Alu = mybir.AluOpType
    TWO_PI = 2.0 * math.pi
    NF = b_core // P128
    assert Pv == 10.0, "wrap/sign tricks assume P == 10"

    delta = TWO_PI / Pv
    th_scale = sharpv / 2.0
    th_bias = -9.0 * sharpv / 2.0

    nc = bacc.Bacc(
        "TRN2", target_bir_lowering=False, debug=False, num_devices=N_CORES
    )
    x_d = nc.dram_tensor("x_dram", [b_core, T], i32, kind="ExternalInput").ap()
    y_d = nc.dram_tensor("y_dram", [b_core, T], i32, kind="ExternalInput").ap()
    lg_d = nc.dram_tensor(
        "logits_dram", [b_core, T, D], fp32, kind="ExternalOutput"
    ).ap()
    cr_d = nc.dram_tensor("carry_dram", [b_core], fp32, kind="ExternalOutput").ap()

    xv = x_d.rearrange("(p n) t -> p (n t)", p=P128)
    yv = y_d.rearrange("(p n) t -> p (n t)", p=P128)
    lv = lg_d.rearrange("(p n) t d -> p n t d", p=P128)
    cv = cr_d.rearrange("(p n) -> p n", p=P128)

    nblk = T // tb
    seg = tb * NF  # elements per (d, t-block) segment

    with tile.TileContext(nc) as tc:
        with (
            tc.tile_pool(name="main", bufs=1) as mp,
            tc.tile_pool(name="hp", bufs=2) as hp,
            tc.tile_pool(name="wp", bufs=2) as wp,
            tc.tile_pool(name="lp", bufs=2) as lp,
        ):
            xs = mp.tile([P128, NF * T], i32, tag="xs")
            ys = mp.tile([P128, NF * T], i32, tag="ys")
            nc.sync.dma_start(xs[:], xv)
            nc.sync.dma_start(ys[:], yv)

            # ssumT = x + y as fp32, t-major (strided single-elem writes ~1cyc)
            ssumT = mp.tile([P128, NF * T], fp32, tag="ssumT")
            sT_nt = ssumT[:].rearrange("p (t n) -> p n t", n=NF)
            x3 = xs[:].rearrange("p (n t) -> p n t", t=T)
            y3 = ys[:].rearrange("p (n t) -> p n t", t=T)
            nc.vector.tensor_tensor(sT_nt, x3, y3, Alu.add)

            R = mp.tile([P128, NF * T], fp32, tag="R")

            thb = mp.tile([P128, 1], fp32, tag="thb")
            nc.vector.memset(thb[:], float(th_bias))

            h_prev = hp.tile([P128, NF], fp32, tag="h")
            nc.vector.memset(h_prev[:], -1.0)

            def chain_steps(t0, t1):
                nonlocal h_prev
                for t in range(t0, t1):
                    rsl = slice(t * NF, (t + 1) * NF)
                    nc.vector.scalar_tensor_tensor(
                        R[:, rsl], h_prev[:], 0.5, ssumT[:, rsl],
                        Alu.mult, Alu.add,
                    )
                    h_new = hp.tile([P128, NF], fp32, tag="h")
                    nc.scalar.activation(
                        h_new[:], R[:, rsl], A.Tanh, bias=thb[:],
                        scale=float(th_scale),
                    )
                    h_prev = h_new

            def phase2_block(bi):
                b0 = bi * tb
                rblk = R[:, b0 * NF : (b0 + tb) * NF]  # contiguous (t,n)
                # wraps write the d-interleaved (t,n,d5) tile
                W = wp.tile([P128, 5 * seg], fp32, tag="W")
                W3 = W[:].rearrange("p (q d) -> p q d", d=5)
                for d in range(5):
                    nc.vector.add_range_wrap(
                        W3[:, :, d], rblk, float(-(7.0 + d)), 5.0, 10.0
                    )
                # sin in place over all 5 segments (contiguous)
                nc.scalar.activation(W[:], W[:], A.Sin, bias=0.0, scale=float(delta))
                # +-L scatters: runs-of-5 read -> runs-of-5 write
                # W q-dim is (t,n); L4 needs (n,t): strided view on the read
                Wv = W[:].rearrange("p (t n d) -> p n t d", t=tb, d=5)
                Lt = lp.tile([P128, NF * tb * D], fp32, tag="L")
                L4 = Lt[:].rearrange("p (n t d) -> p n t d", t=tb, d=D)
                nc.gpsimd.tensor_scalar(
                    L4[:, :, :, 0:5], Wv, float(Lv), 0.0, Alu.mult, Alu.add
                )
                nc.gpsimd.tensor_scalar(
                    L4[:, :, :, 5:10], Wv, float(-Lv), 0.0, Alu.mult, Alu.add
                )
                nc.sync.dma_start(lv[:, :, b0 : b0 + tb, :], L4)

            # interleave: chain block bi+1 emitted after phase2(bi) is queued
            for bi in range(nblk):
                chain_steps(bi * tb, (bi + 1) * tb)
                phase2_block(bi)

            cfin = hp.tile([P128, NF], fp32, tag="cfin")
            nc.vector.tensor_scalar(cfin[:], h_prev[:], 0.5, 0.5, Alu.mult, Alu.add)
            nc.sync.dma_start(cv, cfin[:])

    nc.compile()
    return nc


def kernel(x_digits_rev, y_digits_rev, P, sharp, logit_scale):
    from concourse import bass_utils

    x = np.ascontiguousarray(np.asarray(x_digits_rev), dtype=np.int32)
    y = np.ascontiguousarray(np.asarray(y_digits_rev), dtype=np.int32)
    Pv = float(np.asarray(P))
    sv = float(np.asarray(sharp))
    Lv = float(np.asarray(logit_scale))
    key = (Pv, sv, Lv)
    if key not in _cache:
        _cache[key] = build(Pv, sv, Lv)
    nc = _cache[key]
    in_maps = [
        {
            "x_dram": np.ascontiguousarray(x[c * B_CORE : (c + 1) * B_CORE]),
            "y_dram": np.ascontiguousarray(y[c * B_CORE : (c + 1) * B_CORE]),
        }
        for c in range(N_CORES)
    ]
    res = bass_utils.run_bass_kernel_spmd(nc, in_maps, core_ids=list(range(N_CORES)))
    logits = np.concatenate(
        [res.results[c]["logits_dram"] for c in range(N_CORES)], axis=0
    )
    carry = np.concatenate(
        [res.results[c]["carry_dram"] for c in range(N_CORES)], axis=0
    )
    return logits, carry


# revision 11
# speedup vs baseline: 1.2803x; 1.2803x over previous
import math
import sys

sys.path.insert(0, "/opt/trn_rl_repo")

import numpy as np

N_CORES = 8
B, T, D = 65536, 64, 10
B_CORE = B // N_CORES  # 8192
P128 = 128

_cache = {}


def _patch_act_tables():
    """Force the ACT table pass to pick `silu_and_others` (contains both Sin
    and Tanh). By default the per-func first-match choice alternates
    trig_and_small / sigmoid_and_others, costing a ~1.5us table load per
    Sin<->Tanh transition. Emptying every other set (indices preserved)
    leaves the pass exactly one valid choice for both funcs."""
    import concourse.bacc as bacc
    import concourse.hw_specs as hw_specs

    orig = hw_specs.get_activation_tables

    def patched(arch):
        tabs = orig(arch)
        return {
            name: (funcs if name == "silu_and_others" else set())
            for name, funcs in tabs.items()
        }

    bacc.get_activation_tables = patched


def build(Pv, sharpv, Lv, b_core=B_CORE, tb=16):
    """Build + compile the per-core SPMD Bass program.

    Math: s_t = x_t + y_t, carry c_t (c_0 = 0), u_t = s_t + c_t:
      c_{t+1} = sigmoid(sharp*(u_t - 9.5))
      logits[t,d] = L*cos((2pi/P)*(u_t - d))

    On-chip state: h_t = tanh(sharp*(u_t-9.5)/2) = 2*c_{t+1}-1 (Tanh+Sin
    share the silu_and_others ACT table set) and r_t = s_t + 0.5*h_{t-1}
    = u_t - 0.5:
      r_t   = 0.5*h_{t-1} + s_t                  (one fused DVE op)
      h_t   = Tanh(r_t; scale=sharp/2, bias=-4.5*sharp)
      w_d   = wrap(r - (7+d)) into [-5,5] by +-10  (one ADD_RANGE_WRAP per
              d in [0,5); digit units, so delta*w_d lands in sin's [-pi,pi])
      logits[t,d]   = L*sin(delta*w_d)     == L*cos(delta*(u-d))  exactly
      logits[t,d+5] = -logits[t,d]         (5*delta = pi for P=10)

    Measured HW rules honored here: never use op1=bypass (~14cyc); DVE
    strided single-elem writes ~1cyc, strided reads ~0.9cyc, contiguous
    ts ~0.5cyc; ACT needs contiguous in/out; GpSimd offload poisons DVE
    via the shared SBUF port pair, so the hot path is DVE+ACT only.
    """
    import concourse.bacc as bacc
    import concourse.mybir as mybir
    import concourse.tile as tile

    _patch_act_tables()

    fp32 = mybir.dt.float32
    i32 = mybir.dt.int32
    A = mybir.ActivationFunctionType
    Alu = mybir.AluOpType
    TWO_PI = 2.0 * math.pi
    NF = b_core // P128
    assert Pv == 10.0, "wrap/sign tricks assume P == 10"

    delta = TWO_PI / Pv
    th_scale = sharpv / 2.0
    th_bias = -9.0 * sharpv / 2.0

    nc = bacc.Bacc(
        "TRN2", target_bir_lowering=False, debug=False, num_devices=N_CORES
    )
    x_d = nc.dram_tensor("x_dram", [b_core, T], i32, kind="ExternalInput").ap()
    y_d = nc.dram_tensor("y_dram", [b_core, T], i32, kind="ExternalInput").ap()
    lg_d = nc.dram_tensor(
        "logits_dram", [b_core, T, D], fp32, kind="ExternalOutput"
    ).ap()
    cr_d = nc.dram_tensor("carry_dram", [b_core], fp32, kind="ExternalOutput").ap()

    xv = x_d.rearrange("(p n) t -> p (n t)", p=P128)
    yv = y_d.rearrange("(p n) t -> p (n t)", p=P128)
    lv = lg_d.rearrange("(p n) t d -> p n t d", p=P128)
    cv = cr_d.rearrange("(p n) -> p n", p=P128)

    nblk = T // tb
    seg = tb * NF  # elements per (d, t-block) segment

    with tile.TileContext(nc) as tc:
        with (
            tc.tile_pool(name="main", bufs=1) as mp,
            tc.tile_pool(name="hp", bufs=2) as hp,
            tc.tile_pool(name="wp", bufs=2) as wp,
            tc.tile_pool(name="lp", bufs=2) as lp,
        ):
            xs = mp.tile([P128, NF * T], i32, tag="xs")
            ys = mp.tile([P128, NF * T], i32, tag="ys")
            nc.sync.dma_start(xs[:], xv)
            nc.sync.dma_start(ys[:], yv)

            # ssum = x + y as fp32, (n,t)-contiguous
            ssum = mp.tile([P128, NF * T], fp32, tag="ssum")
            nc.vector.tensor_tensor(ssum[:], xs[:], ys[:], Alu.add)
            ssum3 = ssum[:].rearrange("p (n t) -> p n t", t=T)

            # R is t-major: chain writes contiguous [128, NF] slices
            R = mp.tile([P128, NF * T], fp32, tag="R")

            thb = mp.tile([P128, 1], fp32, tag="thb")
            nc.vector.memset(thb[:], float(th_bias))

            h_prev = hp.tile([P128, NF], fp32, tag="h")
            nc.vector.memset(h_prev[:], -1.0)

            def chain_steps(t0, t1):
                nonlocal h_prev
                for t in range(t0, t1):
                    rsl = slice(t * NF, (t + 1) * NF)
                    nc.vector.scalar_tensor_tensor(
                        R[:, rsl], h_prev[:], 0.5, ssum3[:, :, t],
                        Alu.mult, Alu.add,
                    )
                    h_new = hp.tile([P128, NF], fp32, tag="h")
                    nc.scalar.activation(
                        h_new[:], R[:, rsl], A.Tanh, bias=thb[:],
                        scale=float(th_scale),
                    )
                    h_prev = h_new

            def phase2_block(bi):
                b0 = bi * tb
                rblk = R[:, b0 * NF : (b0 + tb) * NF]  # contiguous (t,n)
                W = wp.tile([P128, 5 * seg], fp32, tag="W")  # d-major
                for d in range(5):
                    nc.vector.add_range_wrap(
                        W[:, d * seg : (d + 1) * seg], rblk,
                        float(-(7.0 + d)), 5.0, 10.0,
                    )
                nc.scalar.activation(W[:], W[:], A.Sin, bias=0.0, scale=float(delta))
                # +-L scatters: strided (n,t,d5) read -> runs-of-5 write
                Wv = W[:].rearrange("p (d t n) -> p n t d", d=5, t=tb)
                Lt = lp.tile([P128, NF * tb * D], fp32, tag="L")
                L4 = Lt[:].rearrange("p (n t d) -> p n t d", t=tb, d=D)
                nc.vector.tensor_scalar(
                    L4[:, :, :, 0:5], Wv, float(Lv), 0.0, Alu.mult, Alu.add
                )
                nc.vector.tensor_scalar(
                    L4[:, :, :, 5:10], Wv, float(-Lv), 0.0, Alu.mult, Alu.add
                )
                nc.sync.dma_start(lv[:, :, b0 : b0 + tb, :], L4)

            for bi in range(nblk):
                chain_steps(bi * tb, (bi + 1) * tb)
                phase2_block(bi)

            cfin = hp.tile([P128, NF], fp32, tag="cfin")
            nc.vector.tensor_scalar(cfin[:], h_prev[:], 0.5, 0.5, Alu.mult, Alu.add)
            nc.sync.dma_start(cv, cfin[:])

    nc.compile()
    return nc


def kernel(x_digits_rev, y_digits_rev, P, sharp, logit_scale):
    from concourse import bass_utils

    x = np.ascontiguousarray(np.asarray(x_digits_rev), dtype=np.int32)
    y = np.ascontiguousarray(np.asarray(y_digits_rev), dtype=np.int32)
    Pv = float(np.asarray(P))
    sv = float(np.asarray(sharp))
    Lv = float(np.asarray(logit_scale))
    key = (Pv, sv, Lv)
    if key not in _cache:
        _cache[key] = build(Pv, sv, Lv)
    nc = _cache[key]
    in_maps = [
        {
            "x_dram": np.ascontiguousarray(x[c * B_CORE : (c + 1) * B_CORE]),
            "y_dram": np.ascontiguousarray(y[c * B_CORE : (c + 1) * B_CORE]),
        }
        for c in range(N_CORES)
    ]
    res = bass_utils.run_bass_kernel_spmd(nc, in_maps, core_ids=list(range(N_CORES)))
    logits = np.concatenate(
        [res.results[c]["logits_dram"] for c in range(N_CORES)], axis=0
    )
    carry = np.concatenate(
        [res.results[c]["carry_dram"] for c in range(N_CORES)], axis=0
    )
    return logits, carry
